# revision 14
# baseline (speedup 1.0000x reference)
"""Trainium2 Bass kernel for nn_MetricNet (512-step elementwise Euler recurrence).

Strategy: pure data parallel over the batch axis — each of the 8 NeuronCores
gets 16384 frequencies laid out as a [128 partitions x 128 free] f32 tile that
lives in SBUF for the whole 512-step recurrence.

Per-step math is reduced to 6 DVE ops + 1 ACT square + 1 GPSIMD op by
  - keeping the Re-state unscaled and shifted (U = Re + inv1), so the U-update
    is a scalar add that the GPSIMD engine handles off the critical path,
  - scaling the Im-state by m = 2*dz*omega (Y = m*Im), which turns every
    per-element coefficient into either a global scalar or the fixed tensor
    W = m^2/2,
  - computing the quadratic bracket via squares: Y^2/2 on the Activation
    engine (free input affine: square(Y/sqrt2)), v^2 on DVE.

    step j (per-step host scalars c1, kt, ktd, S, sigma):
      T1 = (Y + c1)*U        [DVE]   vt = T1 + kt             [DVE]
      Un = T1 + ktd          [GP]    a2 = square(Y/sqrt2)     [ACT]
      v2 = vt*vt             [DVE]   gg = (v2 - S)*W          [DVE]
      Pv = (a2 + sigma) - gg [DVE]   Y' = Y*c1 + Pv           [DVE]

All per-step scalars are host-precomputed in float64 from B and PiT and baked
as fp32 immediates.
"""

import numpy as np

import concourse.bass as bass
import concourse.mybir as mybir
import bass_rust as _br
from concourse import tile
from concourse.bass_utils import run_bass_kernel_spmd

# walrus's codegen rejects instructions carrying more than ~2 sync-wait
# commands, but Tile's exit path hangs the full end-of-kernel wait set
# (one per engine/DMA lane used) on a single SP drain. Split those waits
# across dedicated one-wait NOPs ahead of a bare drain instead.
_orig_drain_and_barrier = tile.TileContext._drain_and_barrier


def _split_drain_and_barrier(self, tick_clock, wait_clock):
    nc = self.nc
    probe = nc.sync.nop()
    wait_clock.add_sem_waits(
        probe.ins, _br.ScopedClock({None: tick_clock.global_clock})
    )
    si = probe.ins.sync_info
    if si is not None and len(si.on_wait) > 1:
        waits = list(si.on_wait)
        probe.ins.sync_info = _br.SyncInfo(
            on_wait=waits[:1], on_update=list(si.on_update)
        )
        for w in waits[1:]:
            extra = nc.sync.nop()
            extra.ins.sync_info = _br.SyncInfo(on_wait=[w], on_update=[])
    nc.sync.drain()
    nc.all_engine_barrier()
    popped = nc._tile_sem_poison_stack.pop()
    assert popped is self._sem_poison
    nc.clear_and_free_semaphores(list(self.sems.allocated().values()))
    nc.all_engine_barrier()


tile.TileContext._drain_and_barrier = _split_drain_and_barrier


def _hoist_extra_waits(nc):
    """walrus's per-instruction sync-wait budget is 1 for compute/DMA
    instructions (2 for TPB_CTRL). Hoist surplus waits onto same-engine NOPs
    spliced immediately before the over-budget instruction — the engine
    executes in order, so waiting earlier is semantically identical."""
    for bb in nc.main_func.blocks:
        insts = bb.instructions
        out = []
        changed = False
        for ins in insts:
            si = ins.sync_info
            if si is not None and len(si.on_wait) > 1:
                waits = list(si.on_wait)
                for w in waits[:-1]:
                    nop = mybir.InstNoOp(
                        name=nc.get_next_instruction_name(),
                        engine=ins.engine,
                        sync_info=_br.SyncInfo(on_wait=[w], on_update=[]),
                    )
                    nc.register_instruction(nop)
                    out.append(nop)
                ins.sync_info = _br.SyncInfo(
                    on_wait=waits[-1:], on_update=list(si.on_update)
                )
                changed = True
            out.append(ins)
        if changed:
            bb.instructions = out


N_LAYERS = 512
Z_INI = 0.0
DEL_Z = 0.9 / 512.0
MU = 1.0
BATCH = 131072
N_CORES = 8
P = 128
F = BATCH // N_CORES // P  # 128

F32 = mybir.dt.float32
ALU = mybir.AluOpType
R2 = float(1.0 / np.sqrt(2.0))


def _host_scalars(B: np.ndarray, p: float):
    """Per-step scalar schedule, float64."""
    zs = Z_INI + DEL_Z * np.arange(N_LAYERS, dtype=np.float64)
    b1 = B.astype(np.float64)[:N_LAYERS]
    b2 = B.astype(np.float64)[1 : N_LAYERS + 1]
    g = 1.0 - b2 / b1
    c1 = 1.0 + g
    inv1 = 1.0 / (p * (1.0 - zs))
    inv2 = inv1 / (1.0 - zs)
    kt = -DEL_Z * inv2
    delta = np.empty(N_LAYERS)
    delta[:-1] = inv1[1:] - inv1[:-1]
    delta[-1] = -inv1[-1]  # so that the final Un = Re_final exactly
    S = -inv2 / p + inv1**2 + 1.0 / b1**2
    T = DEL_Z * zs**2 * (MU * MU) / b1
    sigma = -2.0 * DEL_Z * T
    return c1, kt, delta, S, sigma, inv1


def _build_bass(c1, kt, delta, S, sigma, inv1_0):
    nc = bass.Bass()
    # packed input: [re | im | om] along the free axis; packed output: [re | im]
    x_in = nc.dram_tensor("x_in", [P, 3 * F], F32, kind="ExternalInput")
    x_out = nc.dram_tensor("x_out", [P, 2 * F], F32, kind="ExternalOutput")

    f = float  # immediates
    with tile.TileContext(nc) as tc:
        with tc.tile_pool(name="pool", bufs=1) as pool:
            xin = pool.tile([P, 3 * F], F32)
            nc.gpsimd.dma_start(xin[:], x_in[:])
            re = xin[:, 0:F]
            im = xin[:, F : 2 * F]
            om = xin[:, 2 * F : 3 * F]

            m = pool.tile([P, F], F32)
            W = pool.tile([P, F], F32)
            Ua = pool.tile([P, F], F32)
            Ub = pool.tile([P, F], F32)
            Ya = pool.tile([P, F], F32)
            Yb = pool.tile([P, F], F32)
            T1 = pool.tile([P, F], F32)
            vt = pool.tile([P, F], F32)
            a2 = pool.tile([P, F], F32)
            v2 = pool.tile([P, F], F32)
            gg = pool.tile([P, F], F32)
            Pv = pool.tile([P, F], F32)
            minv = pool.tile([P, F], F32)
            xout = pool.tile([P, 2 * F], F32)
            reo = xout[:, 0:F]
            imo = xout[:, F : 2 * F]

            v = nc.vector
            stt = v.scalar_tensor_tensor
            # m = 2*dz*omega ; W = m*m/2 ; U0 = re + inv1_0 ; Y0 = im*m
            v.tensor_scalar_mul(m[:], om, f(2.0 * DEL_Z))
            stt(W[:], m[:], 0.5, m[:], ALU.mult, ALU.mult)
            v.tensor_scalar_add(Ua[:], re, f(inv1_0))
            v.tensor_mul(Ya[:], im, m[:])

            U, Un = Ua, Ub
            Y, Yn = Ya, Yb
            for j in range(N_LAYERS):
                un_dst = reo if j == N_LAYERS - 1 else Un[:]
                stt(T1[:], Y[:], f(c1[j]), U[:], ALU.add, ALU.mult)
                v.tensor_scalar_add(vt[:], T1[:], f(kt[j]))
                nc.gpsimd.tensor_scalar_add(un_dst, T1[:], f(kt[j] + delta[j]))
                nc.scalar.activation(
                    a2[:], Y[:], mybir.ActivationFunctionType.Square, scale=R2
                )
                v.tensor_mul(v2[:], vt[:], vt[:])
                stt(gg[:], v2[:], f(S[j]), W[:], ALU.subtract, ALU.mult)
                stt(Pv[:], a2[:], f(sigma[j]), gg[:], ALU.add, ALU.subtract)
                stt(Yn[:], Y[:], f(c1[j]), Pv[:], ALU.mult, ALU.add)
                U, Un = Un, U
                Y, Yn = Yn, Y

            v.reciprocal(minv[:], m[:])
            v.tensor_mul(imo, Y[:], minv[:])
            nc.sync.dma_start(x_out[:], xout[:])
    _hoist_extra_waits(nc)
    return nc


def kernel(Re_s, Im_s, omega, PiT, B, _trace=False):
    Re_s = np.ascontiguousarray(Re_s, dtype=np.float32)
    Im_s = np.ascontiguousarray(Im_s, dtype=np.float32)
    omega = np.ascontiguousarray(omega, dtype=np.float32)
    p = float(np.asarray(PiT).reshape(-1)[0])
    c1, kt, delta, S, sigma, inv1 = _host_scalars(np.asarray(B), p)

    nc = _build_bass(c1, kt, delta, S, sigma, float(inv1[0]))

    re8 = Re_s.reshape(N_CORES, P, F)
    im8 = Im_s.reshape(N_CORES, P, F)
    om8 = omega.reshape(N_CORES, P, F)
    xin = np.concatenate([re8, im8, om8], axis=2)  # [8, P, 3F]
    in_maps = [{"x_in": np.ascontiguousarray(xin[i])} for i in range(N_CORES)]
    res = run_bass_kernel_spmd(nc, in_maps, list(range(N_CORES)), trace=_trace)
    re_full = np.concatenate(
        [res.results[i]["x_out"][:, 0:F].reshape(-1) for i in range(N_CORES)]
    )
    im_full = np.concatenate(
        [res.results[i]["x_out"][:, F : 2 * F].reshape(-1) for i in range(N_CORES)]
    )
    if _trace:
        kernel.last_results = res
    return re_full.astype(np.float32), im_full.astype(np.float32)


# revision 15
# speedup vs baseline: 1.3304x; 1.3304x over previous
"""Trainium2 Bass kernel for nn_MetricNet (512-step elementwise Euler recurrence).

Strategy: pure data parallel over the batch axis — each of the 8 NeuronCores
gets 16384 frequencies laid out as a [128 partitions x 128 free] f32 tile that
lives in SBUF for the whole 512-step recurrence.

Per-step math is reduced to 6 DVE ops + 2 ACT ops by
  - keeping the Re-state unscaled and shifted (U = Re + inv1), so the U-update
    is a scalar add that the Activation engine handles off the critical path
    (Copy with its free input affine),
  - scaling the Im-state by m = 2*dz*omega (Y = m*Im), which turns every
    per-element coefficient into either a global scalar or the fixed tensor
    W = m^2/2,
  - computing the quadratic bracket via squares: Y^2/2 on the Activation
    engine (free input affine: square(Y/sqrt2)), v^2 on DVE.

    step j (per-step host scalars c1, kt, ktd, S, sigma):
      T1 = (Y + c1)*U        [DVE]   vt = T1 + kt             [DVE]
      Un = T1 + ktd          [ACT]   a2 = square(Y/sqrt2)     [ACT]
      v2 = vt*vt             [DVE]   gg = (v2 - S)*W          [DVE]
      Pv = (a2 + sigma) - gg [DVE]   Y' = Y*c1 + Pv           [DVE]

All per-step scalars are host-precomputed in float64 from B and PiT and baked
as fp32 immediates.
"""

import numpy as np

import concourse.bass as bass
import concourse.mybir as mybir
import bass_rust as _br
from concourse import tile
from concourse.bass_utils import run_bass_kernel_spmd

# walrus's codegen rejects instructions carrying more than ~2 sync-wait
# commands, but Tile's exit path hangs the full end-of-kernel wait set
# (one per engine/DMA lane used) on a single SP drain. Split those waits
# across dedicated one-wait NOPs ahead of a bare drain instead.
_orig_drain_and_barrier = tile.TileContext._drain_and_barrier


def _split_drain_and_barrier(self, tick_clock, wait_clock):
    nc = self.nc
    probe = nc.sync.nop()
    wait_clock.add_sem_waits(
        probe.ins, _br.ScopedClock({None: tick_clock.global_clock})
    )
    si = probe.ins.sync_info
    if si is not None and len(si.on_wait) > 1:
        waits = list(si.on_wait)
        probe.ins.sync_info = _br.SyncInfo(
            on_wait=waits[:1], on_update=list(si.on_update)
        )
        for w in waits[1:]:
            extra = nc.sync.nop()
            extra.ins.sync_info = _br.SyncInfo(on_wait=[w], on_update=[])
    nc.sync.drain()
    nc.all_engine_barrier()
    popped = nc._tile_sem_poison_stack.pop()
    assert popped is self._sem_poison
    nc.clear_and_free_semaphores(list(self.sems.allocated().values()))
    nc.all_engine_barrier()


tile.TileContext._drain_and_barrier = _split_drain_and_barrier


def _hoist_extra_waits(nc):
    """walrus's per-instruction sync-wait budget is 1 for compute/DMA
    instructions (2 for TPB_CTRL). Hoist surplus waits onto same-engine NOPs
    spliced immediately before the over-budget instruction — the engine
    executes in order, so waiting earlier is semantically identical."""
    for bb in nc.main_func.blocks:
        insts = bb.instructions
        out = []
        changed = False
        for ins in insts:
            si = ins.sync_info
            if si is not None and len(si.on_wait) > 1:
                waits = list(si.on_wait)
                for w in waits[:-1]:
                    nop = mybir.InstNoOp(
                        name=nc.get_next_instruction_name(),
                        engine=ins.engine,
                        sync_info=_br.SyncInfo(on_wait=[w], on_update=[]),
                    )
                    nc.register_instruction(nop)
                    out.append(nop)
                ins.sync_info = _br.SyncInfo(
                    on_wait=waits[-1:], on_update=list(si.on_update)
                )
                changed = True
            out.append(ins)
        if changed:
            bb.instructions = out


N_LAYERS = 512
Z_INI = 0.0
DEL_Z = 0.9 / 512.0
MU = 1.0
BATCH = 131072
N_CORES = 8
P = 128
F = BATCH // N_CORES // P  # 128

F32 = mybir.dt.float32
ALU = mybir.AluOpType
R2 = float(1.0 / np.sqrt(2.0))


def _host_scalars(B: np.ndarray, p: float):
    """Per-step scalar schedule, float64."""
    zs = Z_INI + DEL_Z * np.arange(N_LAYERS, dtype=np.float64)
    b1 = B.astype(np.float64)[:N_LAYERS]
    b2 = B.astype(np.float64)[1 : N_LAYERS + 1]
    g = 1.0 - b2 / b1
    c1 = 1.0 + g
    inv1 = 1.0 / (p * (1.0 - zs))
    inv2 = inv1 / (1.0 - zs)
    kt = -DEL_Z * inv2
    delta = np.empty(N_LAYERS)
    delta[:-1] = inv1[1:] - inv1[:-1]
    delta[-1] = -inv1[-1]  # so that the final Un = Re_final exactly
    S = -inv2 / p + inv1**2 + 1.0 / b1**2
    T = DEL_Z * zs**2 * (MU * MU) / b1
    sigma = -2.0 * DEL_Z * T
    return c1, kt, delta, S, sigma, inv1


def _build_bass(c1, kt, delta, S, sigma, inv1_0):
    nc = bass.Bass()
    # packed input: [re | im | om] along the free axis; packed output: [re | im]
    x_in = nc.dram_tensor("x_in", [P, 3 * F], F32, kind="ExternalInput")
    x_out = nc.dram_tensor("x_out", [P, 2 * F], F32, kind="ExternalOutput")

    f = float  # immediates
    with tile.TileContext(nc) as tc:
        with tc.tile_pool(name="pool", bufs=1) as pool:
            xin = pool.tile([P, 3 * F], F32)
            nc.gpsimd.dma_start(xin[:], x_in[:])
            re = xin[:, 0:F]
            im = xin[:, F : 2 * F]
            om = xin[:, 2 * F : 3 * F]

            m = pool.tile([P, F], F32)
            W = pool.tile([P, F], F32)
            Ua = pool.tile([P, F], F32)
            Ub = pool.tile([P, F], F32)
            Ya = pool.tile([P, F], F32)
            Yb = pool.tile([P, F], F32)
            T1 = pool.tile([P, F], F32)
            vt = pool.tile([P, F], F32)
            a2 = pool.tile([P, F], F32)
            v2 = pool.tile([P, F], F32)
            gg = pool.tile([P, F], F32)
            Pv = pool.tile([P, F], F32)
            minv = pool.tile([P, F], F32)
            xout = pool.tile([P, 2 * F], F32)
            reo = xout[:, 0:F]
            imo = xout[:, F : 2 * F]

            v = nc.vector
            stt = v.scalar_tensor_tensor
            # m = 2*dz*omega ; W = m*m/2 ; U0 = re + inv1_0 ; Y0 = im*m
            v.tensor_scalar_mul(m[:], om, f(2.0 * DEL_Z))
            stt(W[:], m[:], 0.5, m[:], ALU.mult, ALU.mult)
            v.tensor_scalar_add(Ua[:], re, f(inv1_0))
            v.tensor_mul(Ya[:], im, m[:])

            U, Un = Ua, Ub
            Y, Yn = Ya, Yb
            for j in range(N_LAYERS):
                un_dst = reo if j == N_LAYERS - 1 else Un[:]
                stt(T1[:], Y[:], f(c1[j]), U[:], ALU.add, ALU.mult)
                v.tensor_scalar_add(vt[:], T1[:], f(kt[j]))
                nc.scalar.activation(
                    un_dst,
                    T1[:],
                    mybir.ActivationFunctionType.Copy,
                    bias=f(kt[j] + delta[j]),
                )
                nc.scalar.activation(
                    a2[:], Y[:], mybir.ActivationFunctionType.Square, scale=R2
                )
                v.tensor_mul(v2[:], vt[:], vt[:])
                stt(gg[:], v2[:], f(S[j]), W[:], ALU.subtract, ALU.mult)
                stt(Pv[:], a2[:], f(sigma[j]), gg[:], ALU.add, ALU.subtract)
                stt(Yn[:], Y[:], f(c1[j]), Pv[:], ALU.mult, ALU.add)
                U, Un = Un, U
                Y, Yn = Yn, Y

            v.reciprocal(minv[:], m[:])
            v.tensor_mul(imo, Y[:], minv[:])
            nc.sync.dma_start(x_out[:], xout[:])
    _hoist_extra_waits(nc)
    return nc


def kernel(Re_s, Im_s, omega, PiT, B, _trace=False):
    Re_s = np.ascontiguousarray(Re_s, dtype=np.float32)
    Im_s = np.ascontiguousarray(Im_s, dtype=np.float32)
    omega = np.ascontiguousarray(omega, dtype=np.float32)
    p = float(np.asarray(PiT).reshape(-1)[0])
    c1, kt, delta, S, sigma, inv1 = _host_scalars(np.asarray(B), p)

    nc = _build_bass(c1, kt, delta, S, sigma, float(inv1[0]))

    re8 = Re_s.reshape(N_CORES, P, F)
    im8 = Im_s.reshape(N_CORES, P, F)
    om8 = omega.reshape(N_CORES, P, F)
    xin = np.concatenate([re8, im8, om8], axis=2)  # [8, P, 3F]
    in_maps = [{"x_in": np.ascontiguousarray(xin[i])} for i in range(N_CORES)]
    res = run_bass_kernel_spmd(nc, in_maps, list(range(N_CORES)), trace=_trace)
    re_full = np.concatenate(
        [res.results[i]["x_out"][:, 0:F].reshape(-1) for i in range(N_CORES)]
    )
    im_full = np.concatenate(
        [res.results[i]["x_out"][:, F : 2 * F].reshape(-1) for i in range(N_CORES)]
    )
    if _trace:
        kernel.last_results = res
    return re_full.astype(np.float32), im_full.astype(np.float32)
